# revision 1
# baseline (speedup 1.0000x reference)
"""MoE (top-2 of 8 experts, SwiGLU) on 8 Trainium2 NeuronCores.

Strategy (expert-parallel, per the sharding hint):
  - Host computes the router (tiny: [2048,1024]@[1024,8]) and the top-2
    dispatch: for each expert e, the list of tokens routed to it and their
    combine weights. This IS the sharding step — each core's input shard is
    "its expert's weights + its expert's tokens".
  - Core e runs the expert MLP for its ~512 tokens:
        hT = w1[e] @ x_eT            (gate/up fused, [4096, C])
        yT = silu(hT_gate) * hT_up   ([2048, C])
        oT = (w2[e] @ yT) * combine  ([1024, C])
    GEMM1 in bf16, GEMM2 in float32r (same PE speed at moving-dim >=256),
    fp32 PSUM accumulation throughout; activations fp32.
  - Host scatter-adds the per-expert outputs back to token order (unshard).

Layouts keep tokens on the PSUM free dim everywhere so no on-device
transposes are needed; weights are pre-transposed on the host.
"""

import sys

sys.path.insert(0, "/opt/trn_rl_repo")

import numpy as np
import ml_dtypes

import concourse.bass as bass  # noqa: F401  (bass must import before tile)
import concourse.tile as tile
from concourse import bacc, mybir
from concourse.bass_utils import run_bass_kernel_spmd

T = 2048
H = 1024
INTER = 2048
E = 8
TOPK = 2
N_CORES = 8
P = 128

DT = mybir.dt.bfloat16
NP_DT = ml_dtypes.bfloat16

# GEMM2 in float32r: full fp32 operands through the PE at bf16 speed
# (1 cycle/row when the moving dim is >=256). Cuts output error from ~4e-3
# to ~3e-3; HW A/B (3 runs, 26 interleaved loop-slope pairs) shows no
# measurable speed difference vs bf16, so accuracy wins the tie.
G2_F32R = True

_PROGRAM_CACHE = {}    # c_total -> compiled Bacc program (reused across calls)

KH = H // P            # 8  k-tiles for GEMM1 (contract over H)
KI = INTER // P        # 16 k-tiles for GEMM2 (contract over INTER)
NPAIR = INTER // P     # 16 gate/up pairs
NH = H // P            # 8  output h-tiles


def _route(x, router_w):
    """Replicates the reference router in fp32 numpy.

    Returns per-expert (token_indices, combine_weights)."""
    gating = (x @ router_w.T).astype(np.float32)              # [T, E]
    m = gating.max(axis=1, keepdims=True)
    p = np.exp(gating - m, dtype=np.float32)
    probs = p / p.sum(axis=1, keepdims=True)
    order = np.argsort(-probs, axis=1, kind="stable")         # ties -> lower idx
    sel = order[:, :TOPK]                                     # [T, K]
    topw = np.take_along_axis(probs, sel, axis=1)             # [T, K]

    idxs, wts = [], []
    for e in range(E):
        m_e = sel == e                                        # [T, K]
        rows = np.nonzero(m_e.any(axis=1))[0]
        idxs.append(rows.astype(np.int64))
        wts.append(topw[m_e].astype(np.float32))              # aligned with rows
    return idxs, wts


def _chunks(c):
    """Split c tokens into near-equal chunks of <=512 (PSUM bank limit).

    Chunks are kept >=256 where possible: below that, float32r matmuls drop
    to 1/4 rate and LDWEIGHTS (~107 ns) stops hiding under the matmul."""
    n = -(-c // 512)
    base = -(-(-(-c // n)) // 4) * 4                          # ceil(c/n) to mult of 4
    sizes = []
    left = c
    for _ in range(n - 1):
        sizes.append(base)
        left -= base
    sizes.append(left)
    return [s for s in sizes if s > 0]


def _build_program(c_total, loop_n=0):
    """One SPMD program: the expert MLP for c_total (padded) tokens.

    loop_n > 0 wraps the body in an on-device For_i loop running it loop_n
    times (used only by the perf harness to measure the per-iteration slope;
    the graded path uses loop_n=0 = straight-line body)."""
    nc = bacc.Bacc("TRN2", target_bir_lowering=False, debug=False,
                   num_devices=N_CORES)
    f32 = mybir.dt.float32
    xt_d = nc.dram_tensor("xt", [H, c_total], DT, kind="ExternalInput").ap()
    w1t_d = nc.dram_tensor("w1t", [H, 2 * INTER], DT, kind="ExternalInput").ap()
    dt2 = mybir.dt.float32r if G2_F32R else DT
    w2t_d = nc.dram_tensor("w2t", [INTER, H], dt2, kind="ExternalInput").ap()
    sc_d = nc.dram_tensor("scale", [P, c_total], f32, kind="ExternalInput").ap()
    out_d = nc.dram_tensor("out", [H, c_total], f32, kind="ExternalOutput").ap()

    chunk_sizes = _chunks(c_total)

    from contextlib import ExitStack
    with tile.TileContext(nc) as tc, ExitStack() as ctx:
        wpool = ctx.enter_context(tc.tile_pool(name="weights", bufs=1))
        xpool = ctx.enter_context(tc.tile_pool(name="xt", bufs=1))
        ypool = ctx.enter_context(tc.tile_pool(name="yt",
                                               bufs=1 if G2_F32R else 2))
        apool = ctx.enter_context(tc.tile_pool(name="act", bufs=2))
        opool = ctx.enter_context(tc.tile_pool(name="ot", bufs=2))
        pgpool = ctx.enter_context(tc.tile_pool(name="psg", bufs=3, space="PSUM"))
        pupool = ctx.enter_context(tc.tile_pool(name="psu", bufs=3, space="PSUM"))
        popool = ctx.enter_context(tc.tile_pool(name="pso", bufs=2, space="PSUM"))

        if loop_n:
            loop = ctx.enter_context(tc.For_i(
                0, loop_n, 1,
                hint_engines=(mybir.EngineType.PE, mybir.EngineType.SP,
                              mybir.EngineType.Activation, mybir.EngineType.DVE)))

        # ---- PE warmup ----
        # ~3.5 us of dependency-free matmuls on an (uninitialized) scratch
        # tile: the PE HAM clock-gate warms to 2.4 GHz during the initial DMA
        # wait instead of throttling the first real matmuls. The product is
        # never read, so garbage input is fine.
        warm_sb = xpool.tile([P, P], DT, tag="warm")
        nc.vector.memset(warm_sb[:, 0:1], 0.0)
        ps_w = popool.tile([P, P], f32, tag="pso", name="ps_warm")
        # 44 matmuls: ~32 run during the HAM cold window (1.2 GHz, ~107 ns
        # each) burning it on garbage, the rest bridge until the first real
        # operands land (~3.9 us); more than ~48 delays the first real matmul.
        for _ in range(44):
            nc.tensor.matmul(ps_w[:], lhsT=warm_sb[:], rhs=warm_sb[:],
                             start=True, stop=True)

        # ---- input loads ----
        # One merged DMA per logical tensor/piece: the HWDGE prep cost is
        # per-instruction (~625 ns, serialized), so many small DMAs stall the
        # PE at startup.
        NW1P = 8
        W1PC = 2 * INTER // NW1P  # 512

        # xt: 3 DMAs (chunk-1 columns first, split by k — they gate the
        # first matmuls)
        xt_t = xpool.tile([P, KH, c_total], DT, tag="xt")
        xt_view = xt_d.rearrange("(k p) c -> p k c", p=P)
        c1 = chunk_sizes[0]
        nc.sync.dma_start(out=xt_t[:, :KH // 2, :c1],
                          in_=xt_view[:, :KH // 2, :c1])
        xt_sb = [xt_t[:, k, :] for k in range(KH)]

        # first 256 cols of w1 for k=0..3 — unblocks the first two pairs
        w1_0a = wpool.tile([P, KH, 2 * P], DT, tag="w1_0a")
        w1_0a_view = w1t_d[:, :2 * P].rearrange("(k p) c -> p k c", p=P)
        nc.sync.dma_start(out=w1_0a[:, :KH // 2, :],
                          in_=w1_0a_view[:, :KH // 2, :])

        # w1t column pieces (each with all 8 k-tiles), in PE consumption
        # order (gate piece p feeds pairs 4p..4p+3, paired with up piece p+4).
        # Piece 0 is split 256/256 so pairs 0-1 can start while 2-3 stream.
        w1_t = {}

        def load_w1_cols(lo, hi, tag):
            t = wpool.tile([P, KH, hi - lo], DT, tag=tag, name=tag)
            nc.sync.dma_start(
                out=t[:], in_=w1t_d[:, lo:hi].rearrange("(k p) c -> p k c", p=P))
            return t

        nc.sync.dma_start(out=xt_t[:, KH // 2:, :c1],
                          in_=xt_view[:, KH // 2:, :c1])
        nc.sync.dma_start(out=w1_0a[:, KH // 2:, :],
                          in_=w1_0a_view[:, KH // 2:, :])
        w1_t["0a"] = w1_0a
        w1_t["0b"] = load_w1_cols(2 * P, W1PC, "w1_0b")
        if c1 < c_total:
            nc.sync.dma_start(out=xt_t[:, :, c1:], in_=xt_view[:, :, c1:])
        for piece in (4, 1, 5, 2, 6, 3, 7):
            w1_t[piece] = load_w1_cols(piece * W1PC, (piece + 1) * W1PC,
                                       f"w1_{piece}")

        # w2t: two merged DMAs (8 k-tiles each)
        w2_sb = []
        for half in range(2):
            t = wpool.tile([P, KI // 2, H], dt2, tag=f"w2_{half}")
            rs = slice(half * INTER // 2, (half + 1) * INTER // 2)
            nc.sync.dma_start(
                out=t[:], in_=w2t_d[rs, :].rearrange("(k p) c -> p k c", p=P))
            w2_sb.extend(t[:, k, :] for k in range(KI // 2))

        sc_sb = xpool.tile([P, c_total], f32, tag="sc")
        nc.sync.dma_start(out=sc_sb[:], in_=sc_d[:])

        def w1_slice(k, i):
            # stationary lhsT [P(h), P(inter)] for global inter tile i (0..31)
            piece, sub = divmod(i, W1PC // P)
            if piece == 0:
                if sub < 2:
                    return w1_t["0a"][:, k, P * sub:P * (sub + 1)]
                return w1_t["0b"][:, k, P * (sub - 2):P * (sub - 1)]
            return w1_t[piece][:, k, P * sub:P * (sub + 1)]

        # chunk slices (over the token free dim; PSUM caps a chunk at 512)
        csls = []
        c0 = 0
        for cn in chunk_sizes:
            csls.append((slice(c0, c0 + cn), cn))
            c0 += cn

        # ---- GEMM1 + SwiGLU: yT[i] = silu(gate_i) * up_i, [P, c_total] ----
        # Chunk loop is innermost so each w1 stationary tile is consumed
        # across the full GEMM1 span (halves the required w1 DMA bandwidth).
        # Quad structure (4 gate pairs, then their 4 ups) gives the PE ~8 us
        # of gate work from w1 piece p while up piece p+4 is still in flight.
        yt_sb = [None] * NPAIR
        for q in range(NPAIR // 4):
            quad = range(4 * q, 4 * q + 4)
            sgs = {}
            for i in quad:
                yt_sb[i] = ypool.tile([P, c_total], dt2, tag=f"yt{i}",
                                      name=f"yt{i}")
            for ci, (csl, cn) in enumerate(csls):
                for i in quad:
                    ps_g = pgpool.tile([P, cn], f32, tag="psg")
                    for k in range(KH):
                        nc.tensor.matmul(ps_g[:], lhsT=w1_slice(k, i),
                                         rhs=xt_sb[k][:, csl],
                                         start=(k == 0), stop=(k == KH - 1))
                    sg = apool.tile([P, cn], f32, tag=f"sg{i % 4}_{ci}")
                    nc.scalar.activation(sg[:], ps_g[:],
                                         mybir.ActivationFunctionType.Silu)
                    sgs[(i, ci)] = sg
            for ci, (csl, cn) in enumerate(csls):
                for i in quad:
                    ps_u = pupool.tile([P, cn], f32, tag="psu")
                    for k in range(KH):
                        nc.tensor.matmul(ps_u[:], lhsT=w1_slice(k, i + NPAIR),
                                         rhs=xt_sb[k][:, csl],
                                         start=(k == 0), stop=(k == KH - 1))
                    nc.vector.tensor_mul(yt_sb[i][:, csl], sgs[(i, ci)][:],
                                         ps_u[:])

        # ---- GEMM2 + combine scale ----
        for j in range(NH):
            for csl, cn in csls:
                ps_o = popool.tile([P, cn], f32, tag="pso")
                for k in range(KI):
                    nc.tensor.matmul(ps_o[:], lhsT=w2_sb[k][:, P * j:P * (j + 1)],
                                     rhs=yt_sb[k][:, csl],
                                     start=(k == 0), stop=(k == KI - 1))
                ot = opool.tile([P, cn], f32, tag="ot")
                nc.vector.tensor_mul(ot[:], sc_sb[:, csl], ps_o[:])
                nc.sync.dma_start(out=out_d[P * j:P * (j + 1), csl], in_=ot[:])

    nc.compile()
    return nc


def kernel(hidden_states, w1, w2, router_w):
    x = np.ascontiguousarray(np.asarray(hidden_states, dtype=np.float32)
                             .reshape(T, H))
    w1 = np.asarray(w1, dtype=np.float32)
    w2 = np.asarray(w2, dtype=np.float32)
    router_w = np.asarray(router_w, dtype=np.float32)

    idxs, wts = _route(x, router_w)
    c_total = max(64, -(-max(len(i) for i in idxs) // 2) * 2)

    nc = _PROGRAM_CACHE.get(c_total)
    if nc is None:
        nc = _PROGRAM_CACHE[c_total] = _build_program(c_total)

    xt_f32 = x.T  # [H, T]
    in_maps = []
    for e in range(E):
        n = len(idxs[e])
        xt = np.zeros((H, c_total), dtype=NP_DT)
        xt[:, :n] = xt_f32[:, idxs[e]].astype(NP_DT)
        sc = np.zeros((P, c_total), dtype=np.float32)
        sc[:, :n] = wts[e][None, :]
        in_maps.append({
            "xt": xt,
            "w1t": np.ascontiguousarray(w1[e].T).astype(NP_DT),
            "w2t": np.ascontiguousarray(w2[e].T).astype(
                np.float32 if G2_F32R else NP_DT),
            "scale": sc,
        })

    try:
        res = run_bass_kernel_spmd(nc, in_maps, list(range(N_CORES)))
    except Exception:
        # transient runtime hiccups (e.g. mesh desync on a fresh session)
        # usually clear on retry
        res = run_bass_kernel_spmd(nc, in_maps, list(range(N_CORES)))

    out = np.zeros((T, H), dtype=np.float32)
    for e in range(E):
        n = len(idxs[e])
        if n:
            out[idxs[e]] += res.results[e]["out"][:, :n].T
    return out.reshape(1, T, H)



# revision 4
# speedup vs baseline: 1.3044x; 1.3044x over previous
"""MoE (top-2 of 8 experts, SwiGLU) on 8 Trainium2 NeuronCores.

Strategy (expert-parallel + half-expert load balancing):
  - Host computes the router and the top-2 dispatch (exact fp32 replica of
    the reference), yielding per-expert token lists + combine weights.
  - Each expert's MLP is split into TWO half-inter jobs (inter rows
    [0,1024) and [1024,2048)): a job runs GEMM1 for its half of the
    gate/up rows and GEMM2 contracted over its half of INTER, producing a
    partial output for all of its expert's tokens. The halves are exact
    partial sums, added on the host.
  - The 16 jobs are packed 2-per-core: slot A gets the 8 largest token
    counts (padded to cA=max), slot B the 8 smallest (padded to cB). This
    cuts padded columns/core from 2*max_e to max8+max16 (538+507 vs 1076
    for the key(0) routing) — the PE-bound cost scales with padded cols.
  - Per job: hT = w1h[j] @ x_jT (gate/up fused), yT = silu(g)*u,
    oT = (w2h[j] @ yT) * combine. GEMM1 bf16, GEMM2 float32r (full-fp32
    operands at bf16 PE speed for moving dim >=256), fp32 PSUM.
  - Emission order A-G1, B-G1, A-G2, B-G2 keeps the PE fed across the
    GEMM1->GEMM2 transition (B-G1 hides A's yt latency, and w2 DMAs queue
    after both jobs' w1).

Layouts keep tokens on the PSUM free dim everywhere so no on-device
transposes are needed; weights are pre-transposed on the host.
"""

import sys

sys.path.insert(0, "/opt/trn_rl_repo")

import numpy as np
import ml_dtypes

import concourse.bass as bass  # noqa: F401  (bass must import before tile)
import concourse.tile as tile
from concourse import bacc, mybir
from concourse.bass_utils import run_bass_kernel_spmd

T = 2048
H = 1024
INTER = 2048
IH = INTER // 2        # half-inter per job
E = 8
TOPK = 2
N_CORES = 8
P = 128

DT = mybir.dt.bfloat16
NP_DT = ml_dtypes.bfloat16

# GEMM2 in bf16 (not float32r): the balanced two-job layout needs the ~81KB
# of SBUF that fp32 w2/y tiles would cost, and bf16 halves the w2 DMA.
# Output error rises ~4e-3 vs ~3e-3 — far inside the 2e-2 gate.
G2_F32R = False

_PROGRAM_CACHE = {}    # (cA, cB) -> compiled Bacc program

KH = H // P            # 8  k-tiles for GEMM1 (contract over H)
KI = IH // P           # 8  k-tiles for GEMM2 (contract over half INTER)
NPAIR = IH // P        # 8  gate/up pairs per job
NH = H // P            # 8  output h-tiles


def _route(x, router_w):
    """Replicates the reference router in fp32 numpy.

    Returns per-expert (token_indices, combine_weights)."""
    gating = (x @ router_w.T).astype(np.float32)              # [T, E]
    m = gating.max(axis=1, keepdims=True)
    p = np.exp(gating - m, dtype=np.float32)
    probs = p / p.sum(axis=1, keepdims=True)
    order = np.argsort(-probs, axis=1, kind="stable")         # ties -> lower idx
    sel = order[:, :TOPK]                                     # [T, K]
    topw = np.take_along_axis(probs, sel, axis=1)             # [T, K]

    idxs, wts = [], []
    for e in range(E):
        m_e = sel == e                                        # [T, K]
        rows = np.nonzero(m_e.any(axis=1))[0]
        idxs.append(rows.astype(np.int64))
        wts.append(topw[m_e].astype(np.float32))              # aligned with rows
    return idxs, wts


def _assign_jobs(loads):
    """16 half-expert jobs -> 8 cores x 2 slots.

    Slot A holds the 8 largest jobs (padded to their max), slot B the 8
    smallest. Returns (jobsA, jobsB, cA, cB) where jobs* are lists of
    (expert, half) per core."""
    jobs = sorted(((loads[e], e, h) for e in range(E) for h in range(2)),
                  reverse=True)
    a, b = jobs[:N_CORES], jobs[N_CORES:]
    cA = max(64, -(-a[0][0] // 2) * 2)
    cB = max(64, -(-b[0][0] // 2) * 2)
    jobsA = [(e, h) for (_, e, h) in a]
    jobsB = [(e, h) for (_, e, h) in b]
    return jobsA, jobsB, cA, cB


def _chunks(c):
    """Split c tokens into near-equal chunks of <=512 (PSUM bank limit).

    Chunks are kept >=256 where possible: below that, float32r matmuls drop
    to 1/4 rate and LDWEIGHTS (~107 ns) stops hiding under the matmul."""
    n = -(-c // 512)
    base = -(-(-(-c // n)) // 4) * 4                          # ceil(c/n) to mult of 4
    sizes = []
    left = c
    for _ in range(n - 1):
        sizes.append(base)
        left -= base
    sizes.append(left)
    return [s for s in sizes if s > 0]


def _build_program(cA, cB, loop_n=0):
    """One SPMD program: two half-expert jobs (cA and cB padded tokens).

    loop_n > 0 wraps the body in an on-device For_i loop (used only by the
    perf harness to measure the per-iteration slope)."""
    nc = bacc.Bacc("TRN2", target_bir_lowering=False, debug=False,
                   num_devices=N_CORES)
    f32 = mybir.dt.float32
    dt2 = mybir.dt.float32r if G2_F32R else DT
    cs = {0: cA, 1: cB}
    xt_d, w1t_d, w2t_d, sc_d, out_d = {}, {}, {}, {}, {}
    for j in (0, 1):
        xt_d[j] = nc.dram_tensor(f"xt{j}", [H, cs[j]], DT,
                                 kind="ExternalInput").ap()
        w1t_d[j] = nc.dram_tensor(f"w1t{j}", [H, 2 * IH], DT,
                                  kind="ExternalInput").ap()
        w2t_d[j] = nc.dram_tensor(f"w2t{j}", [IH, H], dt2,
                                  kind="ExternalInput").ap()
        sc_d[j] = nc.dram_tensor(f"scale{j}", [P, cs[j]], f32,
                                 kind="ExternalInput").ap()
        out_d[j] = nc.dram_tensor(f"out{j}", [H, cs[j]], f32,
                                  kind="ExternalOutput").ap()

    from contextlib import ExitStack
    with tile.TileContext(nc) as tc, ExitStack() as ctx:
        wpool = ctx.enter_context(tc.tile_pool(name="weights", bufs=1))
        xpool = ctx.enter_context(tc.tile_pool(name="xt", bufs=1))
        ypool = ctx.enter_context(tc.tile_pool(name="yt", bufs=1))
        apool = ctx.enter_context(tc.tile_pool(name="act", bufs=2))
        opool = ctx.enter_context(tc.tile_pool(name="ot", bufs=2))
        pgpool = ctx.enter_context(tc.tile_pool(name="psg", bufs=3, space="PSUM"))
        pupool = ctx.enter_context(tc.tile_pool(name="psu", bufs=3, space="PSUM"))
        popool = ctx.enter_context(tc.tile_pool(name="pso", bufs=2, space="PSUM"))

        if loop_n:
            loop = ctx.enter_context(tc.For_i(
                0, loop_n, 1,
                hint_engines=(mybir.EngineType.PE, mybir.EngineType.SP,
                              mybir.EngineType.Activation, mybir.EngineType.DVE)))

        # ---- PE warmup ----
        # Dependency-free matmuls on an (uninitialized) scratch tile warm the
        # PE HAM clock-gate to 2.4 GHz during the initial DMA wait.
        warm_sb = xpool.tile([P, P], DT, tag="warm")
        nc.vector.memset(warm_sb[:, 0:1], 0.0)
        ps_w = popool.tile([P, P], f32, tag="pso", name="ps_warm")
        for _ in range(44):
            nc.tensor.matmul(ps_w[:], lhsT=warm_sb[:], rhs=warm_sb[:],
                             start=True, stop=True)

        # ---- input loads ----
        # One merged DMA per logical tensor/piece: HWDGE prep (~625 ns) is
        # per-instruction and serialized, so many small DMAs stall startup.
        # Job 0 chunk-1 / first w1 piece lead (they gate the first matmuls);
        # w2 DMAs queue after both jobs' w1 so GEMM1 weights never wait.
        xt_t, xt_sb, sc_sb, w1p = {}, {}, {}, {}
        chunk_sizes = {j: _chunks(cs[j]) for j in (0, 1)}

        W1PC = 512  # w1 piece: 512 cols (4 pairs' gate or up halves)

        def load_w1_cols(j, lo, hi, tag):
            t = wpool.tile([P, KH, hi - lo], DT, tag=tag, name=tag)
            nc.sync.dma_start(
                out=t[:], in_=w1t_d[j][:, lo:hi].rearrange("(k p) c -> p k c", p=P))
            return t

        # job 0: x chunk-1 (split by k) + first 256 w1 cols first
        xt_t[0] = xpool.tile([P, KH, cA], DT, tag="xt0", name="xt0")
        xv0 = xt_d[0].rearrange("(k p) c -> p k c", p=P)
        c1 = chunk_sizes[0][0]
        nc.sync.dma_start(out=xt_t[0][:, :KH // 2, :c1],
                          in_=xv0[:, :KH // 2, :c1])
        w1_0a = wpool.tile([P, KH, 2 * P], DT, tag="w1_0a", name="w1_0a")
        w1_0a_view = w1t_d[0][:, :2 * P].rearrange("(k p) c -> p k c", p=P)
        nc.sync.dma_start(out=w1_0a[:, :KH // 2, :],
                          in_=w1_0a_view[:, :KH // 2, :])
        nc.sync.dma_start(out=xt_t[0][:, KH // 2:, :c1],
                          in_=xv0[:, KH // 2:, :c1])
        nc.sync.dma_start(out=w1_0a[:, KH // 2:, :],
                          in_=w1_0a_view[:, KH // 2:, :])
        if c1 < cA:
            nc.sync.dma_start(out=xt_t[0][:, :, c1:], in_=xv0[:, :, c1:])

        # w1 pieces in PE consumption order: per job, gate piece p feeds
        # pairs 4p..4p+3 paired with up piece p+2. Job 0 piece 0 is split
        # 256/256 so pairs 0-1 start while 2-3 stream.
        w1p[(0, "0b")] = load_w1_cols(0, 2 * P, W1PC, "w1_0_0b")
        for piece in (2, 1, 3):
            w1p[(0, piece)] = load_w1_cols(0, piece * W1PC, (piece + 1) * W1PC,
                                           f"w1_0_{piece}")
        # job 1 x + w1
        xt_t[1] = xpool.tile([P, KH, cB], DT, tag="xt1", name="xt1")
        nc.sync.dma_start(out=xt_t[1][:],
                          in_=xt_d[1].rearrange("(k p) c -> p k c", p=P))
        for piece in (0, 2, 1, 3):
            w1p[(1, piece)] = load_w1_cols(1, piece * W1PC, (piece + 1) * W1PC,
                                           f"w1_1_{piece}")

        for j in (0, 1):
            xt_sb[j] = [xt_t[j][:, k, :] for k in range(KH)]

        # w2: one merged DMA per job (8 k-tiles each), after all w1
        w2_sb = {}
        for j in (0, 1):
            t = wpool.tile([P, KI, H], dt2, tag=f"w2_{j}", name=f"w2_{j}")
            nc.sync.dma_start(
                out=t[:], in_=w2t_d[j].rearrange("(k p) c -> p k c", p=P))
            w2_sb[j] = [t[:, k, :] for k in range(KI)]

        for j in (0, 1):
            sc_sb[j] = xpool.tile([P, cs[j]], f32, tag=f"sc{j}", name=f"sc{j}")
            nc.sync.dma_start(out=sc_sb[j][:], in_=sc_d[j][:])

        def w1_slice(j, k, i):
            # stationary lhsT [P(h), P(inter)] for job-local inter tile i
            # (0..15: 8 gate then 8 up)
            piece, sub = divmod(i, W1PC // P)
            if j == 0 and piece == 0:
                if sub < 2:
                    return w1_0a[:, k, P * sub:P * (sub + 1)]
                return w1p[(0, "0b")][:, k, P * (sub - 2):P * (sub - 1)]
            return w1p[(j, piece)][:, k, P * sub:P * (sub + 1)]

        csls = {}
        for j in (0, 1):
            csls[j] = []
            c0 = 0
            for cn in chunk_sizes[j]:
                csls[j].append((slice(c0, c0 + cn), cn))
                c0 += cn

        yt_sb = {}

        def gemm1(j):
            # yT[i] = silu(gate_i) * up_i, [P, c] per pair i. Quad structure:
            # 4 gate pairs then their 4 ups so the PE has gate work from w1
            # piece p while up piece p+2 streams.
            yt_sb[j] = [None] * NPAIR
            for q in range(NPAIR // 4):
                quad = range(4 * q, 4 * q + 4)
                sgs = {}
                for i in quad:
                    yt_sb[j][i] = ypool.tile([P, cs[j]], dt2, tag=f"yt{j}_{i}",
                                             name=f"yt{j}_{i}")
                for ci, (csl, cn) in enumerate(csls[j]):
                    for i in quad:
                        ps_g = pgpool.tile([P, cn], f32, tag="psg")
                        for k in range(KH):
                            nc.tensor.matmul(ps_g[:], lhsT=w1_slice(j, k, i),
                                             rhs=xt_sb[j][k][:, csl],
                                             start=(k == 0), stop=(k == KH - 1))
                        sg = apool.tile([P, cn], f32, tag=f"sg{i % 4}_{ci}")
                        nc.scalar.activation(sg[:], ps_g[:],
                                             mybir.ActivationFunctionType.Silu)
                        sgs[(i, ci)] = sg
                for ci, (csl, cn) in enumerate(csls[j]):
                    for i in quad:
                        ps_u = pupool.tile([P, cn], f32, tag="psu")
                        for k in range(KH):
                            nc.tensor.matmul(ps_u[:],
                                             lhsT=w1_slice(j, k, i + NPAIR),
                                             rhs=xt_sb[j][k][:, csl],
                                             start=(k == 0), stop=(k == KH - 1))
                        nc.vector.tensor_mul(yt_sb[j][i][:, csl],
                                             sgs[(i, ci)][:], ps_u[:])

        def gemm2(j):
            for jh in range(NH):
                for csl, cn in csls[j]:
                    ps_o = popool.tile([P, cn], f32, tag="pso")
                    for k in range(KI):
                        nc.tensor.matmul(
                            ps_o[:], lhsT=w2_sb[j][k][:, P * jh:P * (jh + 1)],
                            rhs=yt_sb[j][k][:, csl],
                            start=(k == 0), stop=(k == KI - 1))
                    ot = opool.tile([P, cn], f32, tag="ot")
                    nc.vector.tensor_mul(ot[:], sc_sb[j][:, csl], ps_o[:])
                    nc.sync.dma_start(out=out_d[j][P * jh:P * (jh + 1), csl],
                                      in_=ot[:])

        gemm1(0)
        gemm1(1)
        gemm2(0)
        gemm2(1)

    nc.compile()
    return nc


def _make_in_maps(x, w1, w2, router_w):
    """Route + build per-core input shards. Returns (in_maps, meta) where
    meta = (idxs, jobsA, jobsB, cA, cB) for unsharding."""
    idxs, wts = _route(x, router_w)
    loads = [len(i) for i in idxs]
    jobsA, jobsB, cA, cB = _assign_jobs(loads)

    xt_f32 = x.T  # [H, T]
    cache = {}

    def job_tensors(e, h, c_pad):
        n = len(idxs[e])
        if (e, c_pad) not in cache:
            xt = np.zeros((H, c_pad), dtype=NP_DT)
            xt[:, :n] = xt_f32[:, idxs[e]].astype(NP_DT)
            sc = np.zeros((P, c_pad), dtype=np.float32)
            sc[:, :n] = wts[e][None, :]
            cache[(e, c_pad)] = (xt, sc)
        xt, sc = cache[(e, c_pad)]
        # gate rows [h*IH,(h+1)*IH) and up rows [INTER+h*IH, INTER+(h+1)*IH)
        w1j = np.concatenate([w1[e][h * IH:(h + 1) * IH],
                              w1[e][INTER + h * IH:INTER + (h + 1) * IH]], axis=0)
        w2j = w2[e][:, h * IH:(h + 1) * IH]
        return {
            "xt": xt,
            "w1t": np.ascontiguousarray(w1j.T).astype(NP_DT),
            "w2t": np.ascontiguousarray(w2j.T).astype(
                np.float32 if G2_F32R else NP_DT),
            "scale": sc,
        }

    in_maps = []
    for core in range(N_CORES):
        eA, hA = jobsA[core]
        eB, hB = jobsB[core]
        tA = job_tensors(eA, hA, cA)
        tB = job_tensors(eB, hB, cB)
        in_maps.append({
            "xt0": tA["xt"], "w1t0": tA["w1t"], "w2t0": tA["w2t"],
            "scale0": tA["scale"],
            "xt1": tB["xt"], "w1t1": tB["w1t"], "w2t1": tB["w2t"],
            "scale1": tB["scale"],
        })
    return in_maps, (idxs, jobsA, jobsB, cA, cB)


def kernel(hidden_states, w1, w2, router_w):
    x = np.ascontiguousarray(np.asarray(hidden_states, dtype=np.float32)
                             .reshape(T, H))
    w1 = np.asarray(w1, dtype=np.float32)
    w2 = np.asarray(w2, dtype=np.float32)
    router_w = np.asarray(router_w, dtype=np.float32)

    in_maps, (idxs, jobsA, jobsB, cA, cB) = _make_in_maps(x, w1, w2, router_w)

    nc = _PROGRAM_CACHE.get((cA, cB))
    if nc is None:
        nc = _PROGRAM_CACHE[(cA, cB)] = _build_program(cA, cB)

    try:
        res = run_bass_kernel_spmd(nc, in_maps, list(range(N_CORES)))
    except Exception:
        # transient runtime hiccups (e.g. mesh desync on a fresh session)
        # usually clear on retry
        res = run_bass_kernel_spmd(nc, in_maps, list(range(N_CORES)))

    out = np.zeros((T, H), dtype=np.float32)
    for core in range(N_CORES):
        for slot, (e, _h) in (("out0", jobsA[core]), ("out1", jobsB[core])):
            n = len(idxs[e])
            if n:
                out[idxs[e]] += res.results[core][slot][:, :n].T
    return out.reshape(1, T, H)


# revision 7
# speedup vs baseline: 1.5948x; 1.2227x over previous
"""MoE (top-2 of 8 experts, SwiGLU) on 8 Trainium2 NeuronCores.

Strategy (expert-parallel + half-expert load balancing):
  - Host computes the router and the top-2 dispatch (exact fp32 replica of
    the reference), yielding per-expert token lists + combine weights.
  - Each expert's MLP is split into TWO half-inter jobs (inter rows
    [0,1024) and [1024,2048)): a job runs GEMM1 for its half of the
    gate/up rows and GEMM2 contracted over its half of INTER, producing a
    partial output for all of its expert's tokens. The halves are exact
    partial sums, added on the host.
  - The 16 jobs are packed 2-per-core: slot A gets the 8 largest token
    counts (padded to cA=max), slot B the 8 smallest (padded to cB). This
    cuts padded columns/core from 2*max_e to max8+max16 (538+507 vs 1076
    for the key(0) routing) — the PE-bound cost scales with padded cols.
  - Per job: hT = w1h[j] @ x_jT (gate/up fused), yT = silu(g)*u,
    oT = (w2h[j] @ yT) * combine. GEMM1 bf16, GEMM2 float32r (full-fp32
    operands at bf16 PE speed for moving dim >=256), fp32 PSUM.
  - Emission order A-G1, B-G1, A-G2, B-G2 keeps the PE fed across the
    GEMM1->GEMM2 transition (B-G1 hides A's yt latency, and w2 DMAs queue
    after both jobs' w1).

Layouts keep tokens on the PSUM free dim everywhere so no on-device
transposes are needed; weights are pre-transposed on the host.
"""

import sys

sys.path.insert(0, "/opt/trn_rl_repo")

import numpy as np
import ml_dtypes

import concourse.bass as bass  # noqa: F401  (bass must import before tile)
import concourse.tile as tile
from concourse import bacc, mybir
from concourse.bass_utils import run_bass_kernel_spmd

T = 2048
H = 1024
INTER = 2048
IH = INTER // 2        # half-inter per job
E = 8
TOPK = 2
N_CORES = 8
P = 128

DT = mybir.dt.bfloat16
NP_DT = ml_dtypes.bfloat16

# GEMM2 in bf16 (not float32r): the balanced two-job layout needs the ~81KB
# of SBUF that fp32 w2/y tiles would cost, and bf16 halves the w2 DMA.
# Output error rises ~4e-3 vs ~3e-3 — far inside the 2e-2 gate.
G2_F32R = False

_PROGRAM_CACHE = {}    # (cA, cB) -> compiled Bacc program

KH = H // P            # 8  k-tiles for GEMM1 (contract over H)
KI = IH // P           # 8  k-tiles for GEMM2 (contract over half INTER)
NPAIR = IH // P        # 8  gate/up pairs per job
NH = H // P            # 8  output h-tiles


def _route(x, router_w):
    """Replicates the reference router in fp32 numpy.

    Returns per-expert (token_indices, combine_weights)."""
    gating = (x @ router_w.T).astype(np.float32)              # [T, E]
    m = gating.max(axis=1, keepdims=True)
    p = np.exp(gating - m, dtype=np.float32)
    probs = p / p.sum(axis=1, keepdims=True)
    order = np.argsort(-probs, axis=1, kind="stable")         # ties -> lower idx
    sel = order[:, :TOPK]                                     # [T, K]
    topw = np.take_along_axis(probs, sel, axis=1)             # [T, K]

    idxs, wts = [], []
    for e in range(E):
        m_e = sel == e                                        # [T, K]
        rows = np.nonzero(m_e.any(axis=1))[0]
        idxs.append(rows.astype(np.int64))
        wts.append(topw[m_e].astype(np.float32))              # aligned with rows
    return idxs, wts


def _assign_jobs(loads):
    """16 half-expert jobs -> 8 cores x 2 slots.

    Slot A holds the 8 largest jobs (padded to their max), slot B the 8
    smallest. Returns (jobsA, jobsB, cA, cB) where jobs* are lists of
    (expert, half) per core."""
    jobs = sorted(((loads[e], e, h) for e in range(E) for h in range(2)),
                  reverse=True)
    a, b = jobs[:N_CORES], jobs[N_CORES:]
    cA = max(64, -(-a[0][0] // 2) * 2)
    cB = max(64, -(-b[0][0] // 2) * 2)
    jobsA = [(e, h) for (_, e, h) in a]
    jobsB = [(e, h) for (_, e, h) in b]
    return jobsA, jobsB, cA, cB


def _chunks(c):
    """Split c tokens into near-equal chunks of <=512 (PSUM bank limit).

    Chunks are kept >=256 where possible: below that, float32r matmuls drop
    to 1/4 rate and LDWEIGHTS (~107 ns) stops hiding under the matmul."""
    n = -(-c // 512)
    base = -(-(-(-c // n)) // 4) * 4                          # ceil(c/n) to mult of 4
    sizes = []
    left = c
    for _ in range(n - 1):
        sizes.append(base)
        left -= base
    sizes.append(left)
    return [s for s in sizes if s > 0]


def _build_program(cA, cB, loop_n=0):
    """One SPMD program: two half-expert jobs (cA and cB padded tokens).

    loop_n > 0 wraps the body in an on-device For_i loop (used only by the
    perf harness to measure the per-iteration slope)."""
    nc = bacc.Bacc("TRN2", target_bir_lowering=False, debug=False,
                   num_devices=N_CORES)
    f32 = mybir.dt.float32
    dt2 = mybir.dt.float32r if G2_F32R else DT
    cs = {0: cA, 1: cB}
    xt_d, w1t_d, w2t_d, sc_d, out_d = {}, {}, {}, {}, {}
    for j in (0, 1):
        xt_d[j] = nc.dram_tensor(f"xt{j}", [H, cs[j]], DT,
                                 kind="ExternalInput").ap()
        w1t_d[j] = nc.dram_tensor(f"w1t{j}", [H, 2 * IH], DT,
                                  kind="ExternalInput").ap()
        w2t_d[j] = nc.dram_tensor(f"w2t{j}", [IH, H], dt2,
                                  kind="ExternalInput").ap()
        sc_d[j] = nc.dram_tensor(f"scale{j}", [P, cs[j]], f32,
                                 kind="ExternalInput").ap()
        out_d[j] = nc.dram_tensor(f"out{j}", [H, cs[j]], DT,
                                  kind="ExternalOutput").ap()

    from contextlib import ExitStack
    with tile.TileContext(nc) as tc, ExitStack() as ctx:
        wpool = ctx.enter_context(tc.tile_pool(name="weights", bufs=1))
        xpool = ctx.enter_context(tc.tile_pool(name="xt", bufs=1))
        ypool = ctx.enter_context(tc.tile_pool(name="yt", bufs=1))
        apool = ctx.enter_context(tc.tile_pool(name="act", bufs=2))
        opool = ctx.enter_context(tc.tile_pool(name="ot", bufs=1))
        pgpool = ctx.enter_context(tc.tile_pool(name="psg", bufs=3, space="PSUM"))
        pupool = ctx.enter_context(tc.tile_pool(name="psu", bufs=3, space="PSUM"))
        popool = ctx.enter_context(tc.tile_pool(name="pso", bufs=2, space="PSUM"))

        if loop_n:
            loop = ctx.enter_context(tc.For_i(
                0, loop_n, 1,
                hint_engines=(mybir.EngineType.PE, mybir.EngineType.SP,
                              mybir.EngineType.Activation, mybir.EngineType.DVE)))

        # ---- PE warmup ----
        # Dependency-free matmuls on an (uninitialized) scratch tile warm the
        # PE HAM clock-gate to 2.4 GHz during the initial DMA wait.
        warm_sb = xpool.tile([P, P], DT, tag="warm")
        nc.vector.memset(warm_sb[:, 0:1], 0.0)
        ps_w = popool.tile([P, P], f32, tag="pso", name="ps_warm")
        for _ in range(44):
            nc.tensor.matmul(ps_w[:], lhsT=warm_sb[:], rhs=warm_sb[:],
                             start=True, stop=True)

        # ---- input loads ----
        # One merged DMA per logical tensor/piece: HWDGE prep (~625 ns) is
        # per-instruction and serialized, so many small DMAs stall startup.
        # Job 0 chunk-1 / first w1 piece lead (they gate the first matmuls);
        # w2 DMAs queue after both jobs' w1 so GEMM1 weights never wait.
        xt_t, xt_sb, sc_sb, w1p = {}, {}, {}, {}
        chunk_sizes = {j: _chunks(cs[j]) for j in (0, 1)}

        W1PC = 512  # w1 piece: 512 cols (4 pairs' gate or up halves)

        def load_w1_cols(j, lo, hi, tag):
            t = wpool.tile([P, KH, hi - lo], DT, tag=tag, name=tag)
            nc.sync.dma_start(
                out=t[:], in_=w1t_d[j][:, lo:hi].rearrange("(k p) c -> p k c", p=P))
            return t

        # job 0: x chunk-1 (split by k) + first 256 w1 cols first
        xt_t[0] = xpool.tile([P, KH, cA], DT, tag="xt0", name="xt0")
        xv0 = xt_d[0].rearrange("(k p) c -> p k c", p=P)
        c1 = chunk_sizes[0][0]
        nc.sync.dma_start(out=xt_t[0][:, :KH // 2, :c1],
                          in_=xv0[:, :KH // 2, :c1])
        w1_0a = wpool.tile([P, KH, 2 * P], DT, tag="w1_0a", name="w1_0a")
        w1_0a_view = w1t_d[0][:, :2 * P].rearrange("(k p) c -> p k c", p=P)
        nc.sync.dma_start(out=w1_0a[:, :KH // 2, :],
                          in_=w1_0a_view[:, :KH // 2, :])
        nc.sync.dma_start(out=xt_t[0][:, KH // 2:, :c1],
                          in_=xv0[:, KH // 2:, :c1])
        nc.sync.dma_start(out=w1_0a[:, KH // 2:, :],
                          in_=w1_0a_view[:, KH // 2:, :])
        if c1 < cA:
            nc.sync.dma_start(out=xt_t[0][:, :, c1:], in_=xv0[:, :, c1:])

        # w1 pieces in PE consumption order: per job, gate piece p feeds
        # pairs 4p..4p+3 paired with up piece p+2. Job 0 piece 0 is split
        # 256/256 so pairs 0-1 start while 2-3 stream.
        w1p[(0, "0b")] = load_w1_cols(0, 2 * P, W1PC, "w1_0_0b")
        for piece in (2, 1, 3):
            w1p[(0, piece)] = load_w1_cols(0, piece * W1PC, (piece + 1) * W1PC,
                                           f"w1_0_{piece}")
        # job 1 x + w1
        xt_t[1] = xpool.tile([P, KH, cB], DT, tag="xt1", name="xt1")
        nc.sync.dma_start(out=xt_t[1][:],
                          in_=xt_d[1].rearrange("(k p) c -> p k c", p=P))
        for piece in (0, 2, 1, 3):
            w1p[(1, piece)] = load_w1_cols(1, piece * W1PC, (piece + 1) * W1PC,
                                           f"w1_1_{piece}")

        for j in (0, 1):
            xt_sb[j] = [xt_t[j][:, k, :] for k in range(KH)]

        # w2: one merged DMA per job (8 k-tiles each), after all w1
        w2_sb = {}
        for j in (0, 1):
            t = wpool.tile([P, KI, H], dt2, tag=f"w2_{j}", name=f"w2_{j}")
            nc.sync.dma_start(
                out=t[:], in_=w2t_d[j].rearrange("(k p) c -> p k c", p=P))
            w2_sb[j] = [t[:, k, :] for k in range(KI)]

        for j in (0, 1):
            sc_sb[j] = xpool.tile([P, cs[j]], f32, tag=f"sc{j}", name=f"sc{j}")
            nc.sync.dma_start(out=sc_sb[j][:], in_=sc_d[j][:])

        def w1_slice(j, k, i):
            # stationary lhsT [P(h), P(inter)] for job-local inter tile i
            # (0..15: 8 gate then 8 up)
            piece, sub = divmod(i, W1PC // P)
            if j == 0 and piece == 0:
                if sub < 2:
                    return w1_0a[:, k, P * sub:P * (sub + 1)]
                return w1p[(0, "0b")][:, k, P * (sub - 2):P * (sub - 1)]
            return w1p[(j, piece)][:, k, P * sub:P * (sub + 1)]

        csls = {}
        for j in (0, 1):
            csls[j] = []
            c0 = 0
            for cn in chunk_sizes[j]:
                csls[j].append((slice(c0, c0 + cn), cn))
                c0 += cn

        yt_sb = {}

        def gemm1(j):
            # yT[i] = silu(gate_i) * up_i, [P, c] per pair i. Quad structure:
            # 4 gate pairs then their 4 ups so the PE has gate work from w1
            # piece p while up piece p+2 streams.
            yt_sb[j] = [None] * NPAIR
            for q in range(NPAIR // 4):
                quad = range(4 * q, 4 * q + 4)
                sgs = {}
                for i in quad:
                    yt_sb[j][i] = ypool.tile([P, cs[j]], dt2, tag=f"yt{j}_{i}",
                                             name=f"yt{j}_{i}")
                for ci, (csl, cn) in enumerate(csls[j]):
                    for i in quad:
                        ps_g = pgpool.tile([P, cn], f32, tag="psg")
                        for k in range(KH):
                            nc.tensor.matmul(ps_g[:], lhsT=w1_slice(j, k, i),
                                             rhs=xt_sb[j][k][:, csl],
                                             start=(k == 0), stop=(k == KH - 1))
                        sg = apool.tile([P, cn], f32, tag=f"sg{i % 4}_{ci}")
                        nc.scalar.activation(sg[:], ps_g[:],
                                             mybir.ActivationFunctionType.Silu)
                        sgs[(i, ci)] = sg
                for ci, (csl, cn) in enumerate(csls[j]):
                    for i in quad:
                        ps_u = pupool.tile([P, cn], f32, tag="psu")
                        for k in range(KH):
                            nc.tensor.matmul(ps_u[:],
                                             lhsT=w1_slice(j, k, i + NPAIR),
                                             rhs=xt_sb[j][k][:, csl],
                                             start=(k == 0), stop=(k == KH - 1))
                        nc.vector.tensor_mul(yt_sb[j][i][:, csl],
                                             sgs[(i, ci)][:], ps_u[:])

        def gemm2(j):
            # chunk-outer / h-inner so each chunk's 8 h-tiles land in one
            # [P, NH, cn] slice of o_big and ship as ONE DMA — 2 out DMAs
            # per job instead of 16 keeps the serialized HWDGE prep
            # (~625 ns/instruction) off the critical path.
            o_big = opool.tile([P, NH, cs[j]], DT, tag=f"o{j}", name=f"o{j}")
            ov = out_d[j].rearrange("(h p) c -> p h c", p=P)
            for csl, cn in csls[j]:
                for jh in range(NH):
                    ps_o = popool.tile([P, cn], f32, tag="pso")
                    for k in range(KI):
                        nc.tensor.matmul(
                            ps_o[:], lhsT=w2_sb[j][k][:, P * jh:P * (jh + 1)],
                            rhs=yt_sb[j][k][:, csl],
                            start=(k == 0), stop=(k == KI - 1))
                    nc.vector.tensor_mul(o_big[:, jh, csl], sc_sb[j][:, csl],
                                         ps_o[:])
                nc.sync.dma_start(out=ov[:, :, csl], in_=o_big[:, :, csl])

        gemm1(0)
        gemm1(1)
        gemm2(0)
        gemm2(1)

    nc.compile()
    return nc


def _make_in_maps(x, w1, w2, router_w):
    """Route + build per-core input shards. Returns (in_maps, meta) where
    meta = (idxs, jobsA, jobsB, cA, cB) for unsharding."""
    idxs, wts = _route(x, router_w)
    loads = [len(i) for i in idxs]
    jobsA, jobsB, cA, cB = _assign_jobs(loads)

    xt_f32 = x.T  # [H, T]
    cache = {}

    def job_tensors(e, h, c_pad):
        n = len(idxs[e])
        if (e, c_pad) not in cache:
            xt = np.zeros((H, c_pad), dtype=NP_DT)
            xt[:, :n] = xt_f32[:, idxs[e]].astype(NP_DT)
            sc = np.zeros((P, c_pad), dtype=np.float32)
            sc[:, :n] = wts[e][None, :]
            cache[(e, c_pad)] = (xt, sc)
        xt, sc = cache[(e, c_pad)]
        # gate rows [h*IH,(h+1)*IH) and up rows [INTER+h*IH, INTER+(h+1)*IH)
        w1j = np.concatenate([w1[e][h * IH:(h + 1) * IH],
                              w1[e][INTER + h * IH:INTER + (h + 1) * IH]], axis=0)
        w2j = w2[e][:, h * IH:(h + 1) * IH]
        return {
            "xt": xt,
            "w1t": np.ascontiguousarray(w1j.T).astype(NP_DT),
            "w2t": np.ascontiguousarray(w2j.T).astype(
                np.float32 if G2_F32R else NP_DT),
            "scale": sc,
        }

    in_maps = []
    for core in range(N_CORES):
        eA, hA = jobsA[core]
        eB, hB = jobsB[core]
        tA = job_tensors(eA, hA, cA)
        tB = job_tensors(eB, hB, cB)
        in_maps.append({
            "xt0": tA["xt"], "w1t0": tA["w1t"], "w2t0": tA["w2t"],
            "scale0": tA["scale"],
            "xt1": tB["xt"], "w1t1": tB["w1t"], "w2t1": tB["w2t"],
            "scale1": tB["scale"],
        })
    return in_maps, (idxs, jobsA, jobsB, cA, cB)


def kernel(hidden_states, w1, w2, router_w):
    x = np.ascontiguousarray(np.asarray(hidden_states, dtype=np.float32)
                             .reshape(T, H))
    w1 = np.asarray(w1, dtype=np.float32)
    w2 = np.asarray(w2, dtype=np.float32)
    router_w = np.asarray(router_w, dtype=np.float32)

    in_maps, (idxs, jobsA, jobsB, cA, cB) = _make_in_maps(x, w1, w2, router_w)

    nc = _PROGRAM_CACHE.get((cA, cB))
    if nc is None:
        nc = _PROGRAM_CACHE[(cA, cB)] = _build_program(cA, cB)

    try:
        res = run_bass_kernel_spmd(nc, in_maps, list(range(N_CORES)))
    except Exception:
        # transient runtime hiccups (e.g. mesh desync on a fresh session)
        # usually clear on retry
        res = run_bass_kernel_spmd(nc, in_maps, list(range(N_CORES)))

    out = np.zeros((T, H), dtype=np.float32)
    for core in range(N_CORES):
        for slot, (e, _h) in (("out0", jobsA[core]), ("out1", jobsB[core])):
            n = len(idxs[e])
            if n:
                out[idxs[e]] += res.results[core][slot][:, :n].T
    return out.reshape(1, T, H)


# revision 14
# speedup vs baseline: 1.7062x; 1.0698x over previous
"""MoE (top-2 of 8 experts, SwiGLU) on 8 Trainium2 NeuronCores.

Strategy (expert-parallel + half-expert load balancing):
  - Host computes the router and the top-2 dispatch (exact fp32 replica of
    the reference), yielding per-expert token lists + combine weights.
  - Each expert's MLP is split into TWO half-inter jobs (inter rows
    [0,1024) and [1024,2048)): a job runs GEMM1 for its half of the
    gate/up rows and GEMM2 contracted over its half of INTER, producing a
    partial output for all of its expert's tokens. The halves are exact
    partial sums, added on the host.
  - The 16 jobs are packed 2-per-core: slot A gets the 8 largest token
    counts (padded to cA=max), slot B the 8 smallest (padded to cB). This
    cuts padded columns/core from 2*max_e to max8+max16 (538+507 vs 1076
    for the key(0) routing) — the PE-bound cost scales with padded cols.
  - Per job: hT = w1h[j] @ x_jT (gate/up fused), yT = silu(g)*u,
    oT = (w2h[j] @ yT) * combine. GEMM1 bf16, GEMM2 float32r (full-fp32
    operands at bf16 PE speed for moving dim >=256), fp32 PSUM.
  - Emission order A-G1, B-G1, A-G2, B-G2 keeps the PE fed across the
    GEMM1->GEMM2 transition (B-G1 hides A's yt latency, and w2 DMAs queue
    after both jobs' w1).

Layouts keep tokens on the PSUM free dim everywhere so no on-device
transposes are needed; weights are pre-transposed on the host.
"""

import sys

sys.path.insert(0, "/opt/trn_rl_repo")

import numpy as np
import ml_dtypes

import concourse.bass as bass  # noqa: F401  (bass must import before tile)
import concourse.tile as tile
from concourse import bacc, mybir
from concourse.bass_utils import run_bass_kernel_spmd

T = 2048
H = 1024
INTER = 2048
IH = INTER // 2        # half-inter per job
E = 8
TOPK = 2
N_CORES = 8
P = 128

DT = mybir.dt.bfloat16
NP_DT = ml_dtypes.bfloat16

# GEMM2 in bf16 (not float32r): the balanced two-job layout needs the ~81KB
# of SBUF that fp32 w2/y tiles would cost, and bf16 halves the w2 DMA.
# Output error rises ~4e-3 vs ~3e-3 — far inside the 2e-2 gate.
G2_F32R = False

_PROGRAM_CACHE = {}    # (cA, cB) -> compiled Bacc program

KH = H // P            # 8  k-tiles for GEMM1 (contract over H)
KI = IH // P           # 8  k-tiles for GEMM2 (contract over half INTER)
NPAIR = IH // P        # 8  gate/up pairs per job
NH = H // P            # 8  output h-tiles


def _route(x, router_w):
    """Replicates the reference router in fp32 numpy.

    Returns per-expert (token_indices, combine_weights)."""
    gating = (x @ router_w.T).astype(np.float32)              # [T, E]
    m = gating.max(axis=1, keepdims=True)
    p = np.exp(gating - m, dtype=np.float32)
    probs = p / p.sum(axis=1, keepdims=True)
    order = np.argsort(-probs, axis=1, kind="stable")         # ties -> lower idx
    sel = order[:, :TOPK]                                     # [T, K]
    topw = np.take_along_axis(probs, sel, axis=1)             # [T, K]

    idxs, wts = [], []
    for e in range(E):
        m_e = sel == e                                        # [T, K]
        rows = np.nonzero(m_e.any(axis=1))[0]
        idxs.append(rows.astype(np.int64))
        wts.append(topw[m_e].astype(np.float32))              # aligned with rows
    return idxs, wts


def _assign_jobs(loads):
    """16 half-expert jobs -> 8 cores x 2 slots.

    Slot A holds the 8 largest jobs (padded to their max), slot B the 8
    smallest. Returns (jobsA, jobsB, cA, cB) where jobs* are lists of
    (expert, half) per core."""
    jobs = sorted(((loads[e], e, h) for e in range(E) for h in range(2)),
                  reverse=True)
    a, b = jobs[:N_CORES], jobs[N_CORES:]
    cA = max(64, -(-a[0][0] // 2) * 2)
    cB = max(64, -(-b[0][0] // 2) * 2)
    jobsA = [(e, h) for (_, e, h) in a]
    jobsB = [(e, h) for (_, e, h) in b]
    return jobsA, jobsB, cA, cB


def _chunks(c):
    """Split c tokens into near-equal chunks of <=512 (PSUM bank limit).

    Chunks are kept >=256 where possible: below that, float32r matmuls drop
    to 1/4 rate and LDWEIGHTS (~107 ns) stops hiding under the matmul."""
    n = -(-c // 512)
    base = -(-(-(-c // n)) // 4) * 4                          # ceil(c/n) to mult of 4
    sizes = []
    left = c
    for _ in range(n - 1):
        sizes.append(base)
        left -= base
    sizes.append(left)
    return [s for s in sizes if s > 0]


def _build_program(cA, cB, loop_n=0):
    """One SPMD program: two half-expert jobs (cA and cB padded tokens).

    loop_n > 0 wraps the body in an on-device For_i loop (used only by the
    perf harness to measure the per-iteration slope)."""
    nc = bacc.Bacc("TRN2", target_bir_lowering=False, debug=False,
                   num_devices=N_CORES)
    f32 = mybir.dt.float32
    dt2 = mybir.dt.float32r if G2_F32R else DT
    cs = {0: cA, 1: cB}
    xt_d, w1t_d, w2t_d, sc_d, out_d = {}, {}, {}, {}, {}
    for j in (0, 1):
        xt_d[j] = nc.dram_tensor(f"xt{j}", [H, cs[j]], DT,
                                 kind="ExternalInput").ap()
        w1t_d[j] = nc.dram_tensor(f"w1t{j}", [H, 2 * IH], DT,
                                  kind="ExternalInput").ap()
        w2t_d[j] = nc.dram_tensor(f"w2t{j}", [IH, H], dt2,
                                  kind="ExternalInput").ap()
        sc_d[j] = nc.dram_tensor(f"scale{j}", [P, cs[j]], f32,
                                 kind="ExternalInput").ap()
        out_d[j] = nc.dram_tensor(f"out{j}", [H, cs[j]], DT,
                                  kind="ExternalOutput").ap()

    from contextlib import ExitStack
    with tile.TileContext(nc) as tc, ExitStack() as ctx:
        wpool = ctx.enter_context(tc.tile_pool(name="weights", bufs=1))
        xpool = ctx.enter_context(tc.tile_pool(name="xt", bufs=1))
        ypool = ctx.enter_context(tc.tile_pool(name="yt", bufs=1))
        apool = ctx.enter_context(tc.tile_pool(name="act", bufs=2))
        opool = ctx.enter_context(tc.tile_pool(name="ot", bufs=1))
        pgpool = ctx.enter_context(tc.tile_pool(name="psg", bufs=3, space="PSUM"))
        pupool = ctx.enter_context(tc.tile_pool(name="psu", bufs=3, space="PSUM"))
        popool = ctx.enter_context(tc.tile_pool(name="pso", bufs=2, space="PSUM"))

        if loop_n:
            loop = ctx.enter_context(tc.For_i(
                0, loop_n, 1,
                hint_engines=(mybir.EngineType.PE, mybir.EngineType.SP,
                              mybir.EngineType.Activation, mybir.EngineType.DVE)))

        # ---- PE warmup ----
        # Dependency-free matmuls on an (uninitialized) scratch tile warm the
        # PE HAM clock-gate to 2.4 GHz during the initial DMA wait.
        warm_sb = xpool.tile([P, P], DT, tag="warm")
        nc.vector.memset(warm_sb[:, 0:1], 0.0)
        ps_w = popool.tile([P, P], f32, tag="pso", name="ps_warm")
        for _ in range(54):
            nc.tensor.matmul(ps_w[:], lhsT=warm_sb[:], rhs=warm_sb[:],
                             start=True, stop=True)

        # ---- input loads ----
        # One merged DMA per logical tensor/piece: HWDGE prep (~625 ns) is
        # per-instruction and serialized, so many small DMAs stall startup.
        # Job 0 chunk-1 / first w1 piece lead (they gate the first matmuls);
        # w2 DMAs queue after both jobs' w1 so GEMM1 weights never wait.
        xt_t, xt_sb, sc_sb, w1p = {}, {}, {}, {}
        chunk_sizes = {j: _chunks(cs[j]) for j in (0, 1)}

        W1PC = 512  # w1 piece: 512 cols (4 pairs' gate or up halves)

        def load_w1_cols(j, lo, hi, tag):
            t = wpool.tile([P, KH, hi - lo], DT, tag=tag, name=tag)
            nc.sync.dma_start(
                out=t[:], in_=w1t_d[j][:, lo:hi].rearrange("(k p) c -> p k c", p=P))
            return t

        # job 0: x chunk-1 (split by k) + first 256 w1 cols first
        xt_t[0] = xpool.tile([P, KH, cA], DT, tag="xt0", name="xt0")
        xv0 = xt_d[0].rearrange("(k p) c -> p k c", p=P)
        c1 = chunk_sizes[0][0]
        nc.sync.dma_start(out=xt_t[0][:, :, :c1], in_=xv0[:, :, :c1])
        w1_0a = wpool.tile([P, KH, 2 * P], DT, tag="w1_0a", name="w1_0a")
        w1_0a_view = w1t_d[0][:, :2 * P].rearrange("(k p) c -> p k c", p=P)
        nc.sync.dma_start(out=w1_0a[:], in_=w1_0a_view[:])
        if c1 < cA:
            nc.sync.dma_start(out=xt_t[0][:, :, c1:], in_=xv0[:, :, c1:])

        # w1 pieces in PE consumption order: per job, gate piece p feeds
        # pairs 4p..4p+3 paired with up piece p+2. Job 0 piece 0 is split
        # 256/256 so pairs 0-1 start while 2-3 stream.
        w1p[(0, "0b")] = load_w1_cols(0, 2 * P, W1PC, "w1_0_0b")
        for piece in (2, 1, 3):
            w1p[(0, piece)] = load_w1_cols(0, piece * W1PC, (piece + 1) * W1PC,
                                           f"w1_0_{piece}")
        # job 1 x + w1
        xt_t[1] = xpool.tile([P, KH, cB], DT, tag="xt1", name="xt1")
        nc.sync.dma_start(out=xt_t[1][:],
                          in_=xt_d[1].rearrange("(k p) c -> p k c", p=P))
        for piece in (0, 2, 1, 3):
            w1p[(1, piece)] = load_w1_cols(1, piece * W1PC, (piece + 1) * W1PC,
                                           f"w1_1_{piece}")

        for j in (0, 1):
            xt_sb[j] = [xt_t[j][:, k, :] for k in range(KH)]

        # w2: one merged DMA per job (8 k-tiles each), after all w1
        w2_sb = {}
        for j in (0, 1):
            t = wpool.tile([P, KI, H], dt2, tag=f"w2_{j}", name=f"w2_{j}")
            nc.sync.dma_start(
                out=t[:], in_=w2t_d[j].rearrange("(k p) c -> p k c", p=P))
            w2_sb[j] = [t[:, k, :] for k in range(KI)]

        for j in (0, 1):
            sc_sb[j] = xpool.tile([P, cs[j]], f32, tag=f"sc{j}", name=f"sc{j}")
            nc.sync.dma_start(out=sc_sb[j][:], in_=sc_d[j][:])

        def w1_slice(j, k, i):
            # stationary lhsT [P(h), P(inter)] for job-local inter tile i
            # (0..15: 8 gate then 8 up)
            piece, sub = divmod(i, W1PC // P)
            if j == 0 and piece == 0:
                if sub < 2:
                    return w1_0a[:, k, P * sub:P * (sub + 1)]
                return w1p[(0, "0b")][:, k, P * (sub - 2):P * (sub - 1)]
            return w1p[(j, piece)][:, k, P * sub:P * (sub + 1)]

        csls = {}
        for j in (0, 1):
            csls[j] = []
            c0 = 0
            for cn in chunk_sizes[j]:
                csls[j].append((slice(c0, c0 + cn), cn))
                c0 += cn

        yt_sb = {}

        def gemm1(j):
            # yT[i] = silu(gate_i) * up_i, [P, c] per pair i. Quad structure:
            # 4 gate pairs then their 4 ups so the PE has gate work from w1
            # piece p while up piece p+2 streams.
            yt_sb[j] = [None] * NPAIR
            for q in range(NPAIR // 4):
                quad = range(4 * q, 4 * q + 4)
                sgs = {}
                for i in quad:
                    yt_sb[j][i] = ypool.tile([P, cs[j]], dt2, tag=f"yt{j}_{i}",
                                             name=f"yt{j}_{i}")
                for ci, (csl, cn) in enumerate(csls[j]):
                    for i in quad:
                        ps_g = pgpool.tile([P, cn], f32, tag="psg")
                        for k in range(KH):
                            nc.tensor.matmul(ps_g[:], lhsT=w1_slice(j, k, i),
                                             rhs=xt_sb[j][k][:, csl],
                                             start=(k == 0), stop=(k == KH - 1))
                        sg = apool.tile([P, cn], f32, tag=f"sg{i % 4}_{ci}")
                        nc.scalar.activation(sg[:], ps_g[:],
                                             mybir.ActivationFunctionType.Silu)
                        sgs[(i, ci)] = sg
                for ci, (csl, cn) in enumerate(csls[j]):
                    for i in quad:
                        ps_u = pupool.tile([P, cn], f32, tag="psu")
                        for k in range(KH):
                            nc.tensor.matmul(ps_u[:],
                                             lhsT=w1_slice(j, k, i + NPAIR),
                                             rhs=xt_sb[j][k][:, csl],
                                             start=(k == 0), stop=(k == KH - 1))
                        nc.vector.tensor_mul(yt_sb[j][i][:, csl],
                                             sgs[(i, ci)][:], ps_u[:])

        def gemm2(j, last=False):
            # chunk-outer / h-inner so each chunk's 8 h-tiles land in one
            # [P, NH, cn] slice of o_big and ship as ONE DMA — few out DMAs
            # keep the serialized HWDGE prep (~625 ns/instruction) off the
            # critical path. The very last chunk instead ships per-h-pair
            # (4 DMAs) so the final transfer overlaps the remaining h-groups
            # instead of serializing after the last matmul.
            o_big = opool.tile([P, NH, cs[j]], DT, tag=f"o{j}", name=f"o{j}")
            ov = out_d[j].rearrange("(h p) c -> p h c", p=P)
            n_chunks = len(csls[j])
            for ci, (csl, cn) in enumerate(csls[j]):
                tail = last and ci == n_chunks - 1
                for jh in range(NH):
                    ps_o = popool.tile([P, cn], f32, tag="pso")
                    for k in range(KI):
                        nc.tensor.matmul(
                            ps_o[:], lhsT=w2_sb[j][k][:, P * jh:P * (jh + 1)],
                            rhs=yt_sb[j][k][:, csl],
                            start=(k == 0), stop=(k == KI - 1))
                    if tail:
                        # last chunk: per-h tiles + DMAs so each 128-row
                        # slice ships as soon as its combine-mult lands; the
                        # final exposed tail is one small mult + one small
                        # DMA instead of the whole chunk's.
                        o_h = opool.tile([P, 1, cn], DT, tag=f"oh{jh}",
                                         name=f"oh{j}_{jh}")
                        nc.vector.tensor_mul(o_h[:, 0, :],
                                             sc_sb[j][:, csl], ps_o[:])
                        nc.sync.dma_start(out=ov[:, jh:jh + 1, csl],
                                          in_=o_h[:])
                    else:
                        nc.vector.tensor_mul(o_big[:, jh, csl],
                                             sc_sb[j][:, csl], ps_o[:])
                if not tail:
                    nc.sync.dma_start(out=ov[:, :, csl], in_=o_big[:, :, csl])

        gemm1(0)
        gemm1(1)
        # emit last the job whose final chunk is smallest: the tail
        # (mult + out DMA + drain) scales with the final chunk width
        jlast = 0 if chunk_sizes[0][-1] <= chunk_sizes[1][-1] else 1
        gemm2(1 - jlast)
        gemm2(jlast, last=True)

    nc.compile()
    return nc


def _make_in_maps(x, w1, w2, router_w):
    """Route + build per-core input shards. Returns (in_maps, meta) where
    meta = (idxs, jobsA, jobsB, cA, cB) for unsharding."""
    idxs, wts = _route(x, router_w)
    loads = [len(i) for i in idxs]
    jobsA, jobsB, cA, cB = _assign_jobs(loads)

    xt_f32 = x.T  # [H, T]
    cache = {}

    def job_tensors(e, h, c_pad):
        n = len(idxs[e])
        if (e, c_pad) not in cache:
            xt = np.zeros((H, c_pad), dtype=NP_DT)
            xt[:, :n] = xt_f32[:, idxs[e]].astype(NP_DT)
            sc = np.zeros((P, c_pad), dtype=np.float32)
            sc[:, :n] = wts[e][None, :]
            cache[(e, c_pad)] = (xt, sc)
        xt, sc = cache[(e, c_pad)]
        # gate rows [h*IH,(h+1)*IH) and up rows [INTER+h*IH, INTER+(h+1)*IH)
        w1j = np.concatenate([w1[e][h * IH:(h + 1) * IH],
                              w1[e][INTER + h * IH:INTER + (h + 1) * IH]], axis=0)
        w2j = w2[e][:, h * IH:(h + 1) * IH]
        return {
            "xt": xt,
            "w1t": np.ascontiguousarray(w1j.T).astype(NP_DT),
            "w2t": np.ascontiguousarray(w2j.T).astype(
                np.float32 if G2_F32R else NP_DT),
            "scale": sc,
        }

    in_maps = []
    for core in range(N_CORES):
        eA, hA = jobsA[core]
        eB, hB = jobsB[core]
        tA = job_tensors(eA, hA, cA)
        tB = job_tensors(eB, hB, cB)
        in_maps.append({
            "xt0": tA["xt"], "w1t0": tA["w1t"], "w2t0": tA["w2t"],
            "scale0": tA["scale"],
            "xt1": tB["xt"], "w1t1": tB["w1t"], "w2t1": tB["w2t"],
            "scale1": tB["scale"],
        })
    return in_maps, (idxs, jobsA, jobsB, cA, cB)


def kernel(hidden_states, w1, w2, router_w):
    x = np.ascontiguousarray(np.asarray(hidden_states, dtype=np.float32)
                             .reshape(T, H))
    w1 = np.asarray(w1, dtype=np.float32)
    w2 = np.asarray(w2, dtype=np.float32)
    router_w = np.asarray(router_w, dtype=np.float32)

    in_maps, (idxs, jobsA, jobsB, cA, cB) = _make_in_maps(x, w1, w2, router_w)

    nc = _PROGRAM_CACHE.get((cA, cB))
    if nc is None:
        nc = _PROGRAM_CACHE[(cA, cB)] = _build_program(cA, cB)

    try:
        res = run_bass_kernel_spmd(nc, in_maps, list(range(N_CORES)))
    except Exception:
        # transient runtime hiccups (e.g. mesh desync on a fresh session)
        # usually clear on retry
        res = run_bass_kernel_spmd(nc, in_maps, list(range(N_CORES)))

    out = np.zeros((T, H), dtype=np.float32)
    for core in range(N_CORES):
        for slot, (e, _h) in (("out0", jobsA[core]), ("out1", jobsB[core])):
            n = len(idxs[e])
            if n:
                out[idxs[e]] += res.results[core][slot][:, :n].T
    return out.reshape(1, T, H)
